# revision 2
# baseline (speedup 1.0000x reference)
"""Bass/Trainium2 kernel for nn_LogReg_8151847928094.

out[b] = sum_s w[text[s, b]] + bias   (bag-of-words logistic regression)

Strategy (8 NeuronCores, batch-sharded 2048 -> 8 x 256 columns):
  - Every token t has two candidate placements on the 128 SBUF partitions:
      A: partition t % 128,  table entry t // 128      (782 entries)
      B: partition t // 783, table entry 782 + t % 783 (783 entries)
    The host balances each token's A/B choice so that (1) no (partition,
    batch-column) bucket exceeds Rp tokens and (2) no (partition, entry)
    pair exceeds K occurrences (per-value side quota).
  - SBUF holds the 1565-entry fp16 w-table (bias/S pre-added) replicated
    K times per partition. One gpsimd.local_scatter per core scatters
    table entry (p, k*1565+e) to destination slot col*Rp + r (occurrence
    rank r within the bucket); unused source entries carry index -1 and
    are skipped; untouched destination slots are zero-filled by the op.
  - DVE segmented reduce over Rp-slot runs -> [128, 256]; PE ones-matmul
    reduces partitions -> [1, 256]; Act copies PSUM->SBUF; DMA out.
"""

import sys

sys.path.insert(0, "/opt/trn_rl_repo")

import numpy as np

import concourse.bass as bass
import concourse.bacc as bacc
import concourse.mybir as mybir
import concourse.tile as tile
from concourse.bass_utils import run_bass_kernel_spmd

S = 200
B = 2048
V = 100000
NCORES = 8
BS = B // NCORES  # 256 batch columns per core
P = 128
TA = 782  # A-table entries per partition (ceil(V / 128))
TB = 783  # B-table entries per partition (ceil(V / 128) + 1)
TW = TA + TB  # 1565 table entries per partition
KQ = 3  # per-side value quota -> table replica count
NE_LIMIT = 2046  # local_scatter num_elems hard limit

_prog_cache = {}


def _build_program(NI, NEh, Rp, nsplit, loop_T=None):
    nc = bacc.Bacc("TRN2", target_bir_lowering=False, debug=False)
    idxs_d = nc.declare_dram_parameter(
        "idxs", [P, nsplit * NI], mybir.dt.int16, isOutput=False
    )
    tab_d = nc.declare_dram_parameter("tab", [P, NI], mybir.dt.float16, isOutput=False)
    ones_d = nc.declare_dram_parameter(
        "ones", [P, 1], mybir.dt.float32, isOutput=False
    )
    out_d = nc.declare_dram_parameter("out", [1, BS], mybir.dt.float32, isOutput=True)

    with tile.TileContext(nc) as tc:
        with (
            tc.tile_pool(name="sbuf", bufs=1) as pool,
            tc.tile_pool(name="psum", bufs=1, space="PSUM") as psum_pool,
        ):
            idxs_t = pool.tile([P, nsplit * NI], mybir.dt.int16)
            tab_t = pool.tile([P, NI], mybir.dt.float16)
            dst_t = pool.tile([P, nsplit * NEh], mybir.dt.float16)
            red_t = pool.tile([P, BS], mybir.dt.float32)
            ones_t = pool.tile([P, 1], mybir.dt.float32)
            res_t = pool.tile([1, BS], mybir.dt.float32)
            psum_t = psum_pool.tile([1, BS], mybir.dt.float32)

            def body():
                nc.sync.dma_start(out=idxs_t[:], in_=idxs_d[:])
                nc.sync.dma_start(out=tab_t[:], in_=tab_d[:])
                nc.sync.dma_start(out=ones_t[:], in_=ones_d[:])
                for h in range(nsplit):
                    nc.gpsimd.local_scatter(
                        dst_t[:, h * NEh : (h + 1) * NEh],
                        tab_t[:],
                        idxs_t[:, h * NI : (h + 1) * NI],
                        channels=P,
                        num_elems=NEh,
                        num_idxs=NI,
                    )
                nc.vector.tensor_reduce(
                    out=red_t[:],
                    in_=dst_t[:].rearrange("p (b r) -> p b r", r=Rp),
                    axis=mybir.AxisListType.X,
                    op=mybir.AluOpType.add,
                )
                nc.tensor.matmul(
                    psum_t[:], lhsT=ones_t[:], rhs=red_t[:], start=True, stop=True
                )
                nc.scalar.copy(out=res_t[:], in_=psum_t[:])
                nc.sync.dma_start(out=out_d[:], in_=res_t[:])

            if loop_T is None:
                body()
            else:
                with tc.For_i(0, loop_T, 1) as _i:
                    body()
    nc.compile()
    return nc


def _rank_within(keys):
    """Occurrence rank of each element within its equal-key group."""
    o = np.argsort(keys, kind="stable")
    ks = keys[o]
    st = np.r_[0, np.flatnonzero(np.diff(ks)) + 1]
    rid = np.cumsum(np.r_[0, np.diff(ks) != 0])
    rk = np.empty(keys.size, np.int64)
    rk[o] = np.arange(keys.size) - st[rid]
    return rk


def _prep_core(t2d):
    """tokens [S, BS] -> (p, entry, col, r, k, Rp, K) flat arrays + maxima."""
    s, bs = t2d.shape
    t = t2d.astype(np.int64).ravel(order="F")
    col = np.repeat(np.arange(bs, dtype=np.int64), s)
    pA = t % P
    pB = t // TB
    bidA = pA * bs + col
    bidB = pB * bs + col
    rv = _rank_within(t)
    cnt = np.bincount(t, minlength=V)[t]
    side = np.where(cnt <= KQ, False, (rv % 2).astype(bool))
    for L in (9, 8, 7, 6, 5, 4):
        for _ in range(8):
            cur = np.where(side, bidB, bidA)
            alt = np.where(side, bidA, bidB)
            load = np.bincount(cur, minlength=P * bs)
            rk = _rank_within(cur)
            vs = t * 2 + side
            vcnt = np.bincount(vs, minlength=2 * V)
            altv = t * 2 + (~side)
            cand = (rk >= L) & (load[alt] < L) & (vcnt[altv] < KQ)
            idxs = np.flatnonzero(cand)
            if idxs.size == 0:
                break
            a_rk = _rank_within(alt[idxs])
            v_rk = _rank_within(altv[idxs])
            acc = idxs[
                (load[alt[idxs]] + a_rk < L) & (vcnt[altv[idxs]] + v_rk < KQ)
            ]
            if acc.size == 0:
                break
            side[acc] = ~side[acc]
    cur = np.where(side, bidB, bidA)
    load = np.bincount(cur, minlength=P * bs)
    Rp = int(load.max())
    p = np.where(side, pB, pA)
    entry = np.where(side, TA + t % TB, t // P)
    r = _rank_within(cur)
    k = _rank_within(p * TW + entry)
    K = int(k.max()) + 1
    return p, entry, col, r, k, Rp, K


def _pack_core(prep, NI, Rp, nsplit):
    """Build the [P, nsplit*NI] int16 index array for one core."""
    p, entry, col, r, k = prep
    csz = BS // nsplit  # columns per scatter group
    g = col // csz
    slot = (col % csz) * Rp + r
    idx = np.full((P, nsplit * NI), -1, np.int16)
    idx[p, g * NI + k * TW + entry] = slot.astype(np.int16)
    return idx


def _make_table(w, b, K, NI):
    """[P, NI] fp16 table: K replicas of [A(782) | B(783)], bias/S folded."""
    wpad = np.zeros(P * TB, np.float32)
    wpad[:V] = w + b / S
    tabA = wpad[: TA * P].reshape(TA, P).T  # [p, e] = w[e*128 + p]
    tabB = wpad.reshape(P, TB)  # [p, e] = w[p*783 + e]
    tab = np.concatenate([tabA, tabB], axis=1).astype(np.float16)  # [P, TW]
    rep = np.tile(tab, (1, K))
    if rep.shape[1] < NI:
        rep = np.concatenate(
            [rep, np.zeros((P, NI - rep.shape[1]), np.float16)], axis=1
        )
    return rep


def _plan(text):
    """Host prep shared by kernel() and the timing harness."""
    preps = []
    Rp = K = 0
    for c in range(NCORES):
        p, entry, col, r, k, Rp_c, K_c = _prep_core(text[:, c * BS : (c + 1) * BS])
        preps.append((p, entry, col, r, k))
        Rp = max(Rp, Rp_c)
        K = max(K, K_c)
    nsplit = 1
    while (BS // nsplit) * Rp > NE_LIMIT:
        nsplit *= 2
    NEh = (BS // nsplit) * Rp
    if NEh % 2:
        NEh += 1
    NI = K * TW
    if NI % 2:
        NI += 1
    return preps, NI, NEh, Rp, K, nsplit


def _in_maps(preps, NI, Rp, K, nsplit, w, b):
    tab = _make_table(w, b, K, NI)
    ones = np.ones((P, 1), np.float32)
    maps = []
    for c in range(NCORES):
        idx = _pack_core(preps[c], NI, Rp, nsplit)
        maps.append({"idxs": idx, "tab": tab, "ones": ones})
    return maps


def kernel(text, w, b):
    text = np.asarray(text)
    w = np.asarray(w, dtype=np.float32).reshape(-1)
    b = np.asarray(b, dtype=np.float32).reshape(-1)

    preps, NI, NEh, Rp, K, nsplit = _plan(text)
    key = (NI, NEh, Rp, nsplit)
    nc = _prog_cache.get(key)
    if nc is None:
        nc = _build_program(NI, NEh, Rp, nsplit)
        _prog_cache[key] = nc

    maps = _in_maps(preps, NI, Rp, K, nsplit, w, b[0])
    res = run_bass_kernel_spmd(nc, maps, list(range(NCORES))).results
    out = np.concatenate([res[c]["out"][0] for c in range(NCORES)])
    return out.astype(np.float32)


if __name__ == "__main__":
    rng = np.random.default_rng(0)
    text = rng.integers(0, V, (S, B)).astype(np.int64)
    w = rng.standard_normal((1, V)).astype(np.float32) * 0.01
    b = np.zeros((1,), np.float32)
    out = kernel(text, w, b)
    exp = w[0][text].sum(axis=0) + b[0]
    err = np.abs(out - exp).max() / (np.abs(exp).max() + 1e-9)
    print("rel err:", err)


# revision 9
# speedup vs baseline: 100.1317x; 100.1317x over previous
"""Bass/Trainium2 kernel for nn_LogReg_8151847928094.

out[b] = sum_s w[text[s, b]] + bias   (bag-of-words logistic regression)

Strategy (8 NeuronCores, batch-sharded 2048 -> 8 x 256 columns):
  - Every token t has two candidate placements on the 128 SBUF partitions:
      A: partition t % 128,  table entry t // 128      (782 entries)
      B: partition t // 783, table entry 782 + t % 783 (783 entries)
    The host flips per-token A/B choices so no (partition, batch-column)
    bucket holds more than Rp tokens (excess moved to the lighter side).
  - gpsimd.local_scatter runs with per-partition independent indices
    (-1 skips; the dst window is zero-filled by the op).  Two scatters:
      S0: fp16 w-table (bias/S folded) [128, 1566] -> staging[:U0], one
          slot per used (partition, entry), slots sorted by multiplicity
          descending so the layer prefixes below stay dense.
      D:  staging_cat -> dst[col*Rp + r]: delivers every token
          occurrence.  staging_cat = [staging | staging[:U2] | ...] --
          tiny DVE copies duplicate the high-multiplicity prefix so one
          scatter can deliver the k-th occurrence of each entry.
  - DVE segmented reduce over Rp-runs -> [128, 256]; PE ones-matmul
    reduces partitions -> PSUM; DVE copies to SBUF; DMA out.
  - Timing harness wraps the body in For_i_pipelined (unroll=4): DMA,
    the gpsimd chain, and the reduce tail overlap across iterations.
"""

import sys

sys.path.insert(0, "/opt/trn_rl_repo")

import numpy as np

import concourse.bass as bass
import concourse.bacc as bacc
import concourse.mybir as mybir
import concourse.tile as tile
from concourse import library_config
from concourse.bass_utils import run_bass_kernel_spmd

S = 200
B = 2048
V = 100000
NCORES = 8
BS = B // NCORES  # 256 batch columns per core
P = 128
TA = 782  # A-table entries per partition (ceil(V / 128))
TB = 783  # B-table entries per partition
TW = TA + TB  # 1565 table entries per partition
TWpad = TW + (TW % 2)  # 1566

_prog_cache = {}


# ----------------------------- host packing -----------------------------


def _rank_within(keys):
    """Occurrence rank of each element within its equal-key group."""
    o = np.argsort(keys, kind="stable")
    ks = keys[o]
    st = np.r_[0, np.flatnonzero(np.diff(ks)) + 1]
    rid = np.cumsum(np.r_[0, np.diff(ks) != 0])
    rk = np.empty(keys.size, np.int64)
    rk[o] = np.arange(keys.size) - st[rid]
    return rk


def _balance_core(t2d):
    """Per-token A/B choice: cap (partition, column) bucket loads."""
    s, bs = t2d.shape
    t = t2d.astype(np.int64).ravel(order="F")
    col = np.repeat(np.arange(bs, dtype=np.int64), s)
    pA = t % P
    pB = t // TB
    bidA = pA * bs + col
    bidB = pB * bs + col
    side = (_rank_within(t) % 2).astype(bool)
    for L in (7, 6, 5, 4, 3):
        for _ in range(12):
            cur = np.where(side, bidB, bidA)
            alt = np.where(side, bidA, bidB)
            load = np.bincount(cur, minlength=P * bs)
            cand = (load[cur] > L) & (load[alt] < L)
            idxs = np.flatnonzero(cand)
            if idxs.size == 0:
                break
            # within each overloaded bucket move only the excess,
            # lightest alternative first
            o = np.lexsort((load[alt[idxs]], cur[idxs]))
            oi = idxs[o]
            r_c = _rank_within(cur[oi])
            oi = oi[r_c < (load[cur[oi]] - L)]
            if oi.size == 0:
                break
            a_rk = _rank_within(alt[oi])
            acc = oi[load[alt[oi]] + a_rk < L]
            if acc.size == 0:
                break
            side[acc] = ~side[acc]
    cur = np.where(side, bidB, bidA)
    load = np.bincount(cur, minlength=P * bs)
    p = np.where(side, pB, pA)
    entry = np.where(side, TA + t % TB, t // P)
    return p, entry, col, _rank_within(cur), int(load.max())


def _prep_core(t2d):
    """Balanced placement + staging structure for one core."""
    p, entry, col, r, Rp = _balance_core(t2d)
    gid = p * TW + entry
    uniq, ginv, mult = np.unique(gid, return_inverse=True, return_counts=True)
    grp_p = uniq // TW
    grp_entry = uniq % TW
    # staging slots sorted by multiplicity desc within each partition
    ordg = np.lexsort((np.arange(uniq.size), -mult, grp_p))
    slot_sorted = _rank_within(grp_p[ordg])
    grp_slot = np.empty(uniq.size, np.int64)
    grp_slot[ordg] = slot_sorted
    u_tok = grp_slot[ginv]
    k = _rank_within(gid)  # occurrence layer of each token
    M_c = int(mult.max())
    Uk = [
        int(np.bincount(grp_p[mult >= kk], minlength=P).max())
        for kk in range(1, M_c + 1)
    ]
    return dict(
        p=p, col=col, r=r, k=k, u_tok=u_tok,
        grp_p=grp_p, grp_entry=grp_entry, grp_slot=grp_slot,
        Rp=Rp, M=M_c, Uk=Uk,
    )


def _plan(text):
    """Host prep shared by kernel() and the timing harness."""
    cores = [_prep_core(text[:, c * BS : (c + 1) * BS]) for c in range(NCORES)]
    M = max(c["M"] for c in cores)
    Uk = [max(c["Uk"][kk] if kk < c["M"] else 0 for c in cores) for kk in range(M)]
    Uk = [u + u % 2 for u in Uk]  # each region even-sized
    Rp = max(c["Rp"] for c in cores)
    NE = BS * Rp
    assert NE * 32 < 2**16 and Uk[0] * 32 < 2**16
    SU = sum(Uk)
    return dict(cores=cores, M=M, Uk=Uk, Rp=Rp, NE=NE, SU=SU, NIdx=TWpad + SU)


def _pack_core(ci, plan):
    """Concatenated idx array [P, TWpad + SU] int16 for one core."""
    M, Uk, Rp = plan["M"], plan["Uk"], plan["Rp"]
    idx0 = np.full((P, TWpad), -1, np.int16)
    idx0[ci["grp_p"], ci["grp_entry"]] = ci["grp_slot"].astype(np.int16)
    idxD = np.full((P, plan["SU"]), -1, np.int16)
    p, colr, k = ci["p"], ci["col"], ci["k"]
    off = 0
    for kk in range(M):
        if kk < ci["M"]:
            sel = k == kk
            idxD[p[sel], off + ci["u_tok"][sel]] = (
                colr[sel] * Rp + ci["r"][sel]
            ).astype(np.int16)
        off += Uk[kk]
    return np.concatenate([idx0, idxD], axis=1)


def _make_table(w, b):
    """[P, TWpad] fp16 table [A(782) | B(783)], bias/S folded in."""
    wpad = np.zeros(P * TB, np.float32)
    wpad[:V] = w + b / S
    tabA = wpad[: TA * P].reshape(TA, P).T  # [p, e] = w[e*128 + p]
    tabB = wpad.reshape(P, TB)  # [p, e] = w[p*783 + e]
    tab = np.concatenate([tabA, tabB], axis=1).astype(np.float16)
    if tab.shape[1] < TWpad:
        tab = np.concatenate(
            [tab, np.zeros((P, TWpad - tab.shape[1]), np.float16)], axis=1
        )
    return tab


def _in_maps(plan, w, b):
    tab = _make_table(w, b)
    return [
        {"idxs": _pack_core(plan["cores"][c], plan), "tab": tab}
        for c in range(NCORES)
    ]


# ----------------------------- device program ---------------------------


def _build_program(plan, loop_T=None, unroll=4):
    M, Uk, Rp = plan["M"], plan["Uk"], plan["Rp"]
    NE, SU, NIdx = plan["NE"], plan["SU"], plan["NIdx"]
    U0 = Uk[0]
    nc = bacc.Bacc("TRN2", target_bir_lowering=False, debug=False)
    idxs_d = nc.declare_dram_parameter("idxs", [P, NIdx], mybir.dt.int16, isOutput=False)
    tab_d = nc.declare_dram_parameter("tab", [P, TWpad], mybir.dt.float16, isOutput=False)
    out_d = nc.declare_dram_parameter("out", [1, BS], mybir.dt.float32, isOutput=True)

    with tile.TileContext(nc) as tc:
        with (
            tc.tile_pool(name="sbuf", bufs=1) as pool,
            tc.tile_pool(name="pipe", bufs=unroll) as pipe_pool,
            tc.tile_pool(name="psum", bufs=1, space="PSUM") as psum_pool,
        ):
            ones_t = pool.tile([P, 1], mybir.dt.float32)
            red_t = pool.tile([P, BS], mybir.dt.float32)
            res_t = pool.tile([1, BS], mybir.dt.float32)
            psum_t = psum_pool.tile([1, BS], mybir.dt.float32)

            # One-time init: gpsimd library + structural ones vector.
            nc.gpsimd.load_library(library_config.local_scatter)
            nc.vector.memset(ones_t[:], 1.0)

            def compute_chain(stag_t, dst_t, tab_t, idxs_t):
                nc.gpsimd.local_scatter(
                    stag_t[:, :U0], tab_t[:], idxs_t[:, :TWpad],
                    channels=P, num_elems=U0, num_idxs=TWpad,
                )
                off = U0
                for kk in range(1, M):
                    nc.vector.tensor_scalar_add(
                        stag_t[:, off : off + Uk[kk]], stag_t[:, : Uk[kk]], 0.0
                    )
                    off += Uk[kk]
                nc.gpsimd.local_scatter(
                    dst_t[:], stag_t[:], idxs_t[:, TWpad:],
                    channels=P, num_elems=NE, num_idxs=SU,
                )

            def tail(dst_t):
                nc.vector.tensor_reduce(
                    out=red_t[:],
                    in_=dst_t[:].rearrange("p (b r) -> p b r", r=Rp),
                    axis=mybir.AxisListType.X,
                    op=mybir.AluOpType.add,
                )
                nc.tensor.matmul(
                    psum_t[:], lhsT=ones_t[:], rhs=red_t[:], start=True, stop=True
                )
                nc.vector.tensor_scalar_add(res_t[:], psum_t[:], 0.0)
                nc.sync.dma_start(out=out_d[:], in_=res_t[:])

            if loop_T is None:
                idxs_t = pool.tile([P, NIdx], mybir.dt.int16)
                tab_t = pool.tile([P, TWpad], mybir.dt.float16)
                stag_t = pool.tile([P, SU], mybir.dt.float16)
                dst_t = pool.tile([P, NE], mybir.dt.float16)
                nc.sync.dma_start(out=idxs_t[:], in_=idxs_d[:])
                nc.sync.dma_start(out=tab_t[:], in_=tab_d[:])
                compute_chain(stag_t, dst_t, tab_t, idxs_t)
                tail(dst_t)
            else:

                def load(pipe, iv):
                    idxs_t = pipe.intermediate_tile([P, NIdx], mybir.dt.int16)
                    tab_t = pipe.intermediate_tile([P, TWpad], mybir.dt.float16)
                    nc.sync.dma_start(out=idxs_t[:], in_=idxs_d[:])
                    nc.sync.dma_start(out=tab_t[:], in_=tab_d[:])
                    return idxs_t, tab_t

                def compute(pipe, iv, tiles):
                    idxs_t, tab_t = tiles
                    stag_t = pipe.intermediate_tile([P, SU], mybir.dt.float16)
                    dst_t = pipe.intermediate_tile([P, NE], mybir.dt.float16)
                    compute_chain(stag_t, dst_t, tab_t, idxs_t)
                    return dst_t

                def store(pipe, iv, dst_t):
                    tail(dst_t)

                tc.For_i_pipelined(
                    [load, compute, store], 0, loop_T, unroll=unroll, pool=pipe_pool
                )
    nc.compile()
    return nc


def kernel(text, w, b):
    text = np.asarray(text)
    w = np.asarray(w, dtype=np.float32).reshape(-1)
    b = np.asarray(b, dtype=np.float32).reshape(-1)

    plan = _plan(text)
    key = (plan["M"], tuple(plan["Uk"]), plan["Rp"])
    nc = _prog_cache.get(key)
    if nc is None:
        nc = _build_program(plan)
        _prog_cache[key] = nc

    maps = _in_maps(plan, w, b[0])
    res = run_bass_kernel_spmd(nc, maps, list(range(NCORES))).results
    out = np.concatenate([res[c]["out"][0] for c in range(NCORES)])
    return out.astype(np.float32)


if __name__ == "__main__":
    rng = np.random.default_rng(0)
    text = rng.integers(0, V, (S, B)).astype(np.int64)
    w = rng.standard_normal((1, V)).astype(np.float32) * 0.01
    b = np.zeros((1,), np.float32)
    out = kernel(text, w, b)
    exp = w[0][text].sum(axis=0) + b[0]
    err = np.abs(out - exp).max() / (np.abs(exp).max() + 1e-9)
    print("rel err:", err)
